# revision 1
# baseline (speedup 1.0000x reference)
"""BiAffine attention kernel for Trainium2, 8 NeuronCores.

Problem: b=8, n1=n2=2048, h=1024 (fp32)
  S2_h   = S2 @ W1.T ; scores1 = S1 @ S2_h.T ; attn1 = softmax(scores1) ; O1 = attn1 @ S2
  S1_h   = S1 @ W2.T ; scores2 = S2 @ S1_h.T ; attn2 = softmax(scores2) ; O2 = attn2 @ S1

Reformulated (per batch):
  scores1 = (S1 @ W1) @ S2^T        scores2 = (S2 @ W2) @ S1^T

Sharding: data-parallel over batch, 1 batch per core (8 cores).

Per-core plan (all matmuls fp32r = fp22 multiply, fp32 accumulate):
  T1: transpose S1 -> s1T (SBUF), spill s1T to HBM scratch
  W1: s1wT = W1^T-free matmul: s1wT[h',m] = sum_k W1[k,h'] * s1T[k,m] -> HBM
  T2: load S2 natural (resident) + transpose -> s2T (resident)
  W2: s2wT -> HBM
  A1: attention rows, software-pipelined across 128-row tiles: scores in
      PSUM -> chunked softmax (DVE max, ACT exp+rowsum) for tile mt runs
      while the PE does attn-transposes + AV for tile mt-1; output scaled
      by 1/rowsum on the way out.
  R2: reload s1T + S1 natural
  A2: direction 2, symmetric.

Measured (in-NEFF loop diff, 8 cores parallel): ~0.95 ms per full pass.
"""

import sys

sys.path.insert(0, "/opt/trn_rl_repo")

import numpy as np

import concourse.bass as bass
import concourse.tile as tile
import concourse.mybir as mybir
from concourse import masks
from concourse.vector_clock import ScopedClock
import concourse.bass_utils as _bu

_orig_run_command = _bu.run_command


def _run_command_ldwopt(argv, **kw):
    argv = ["--enable-ldw-opt=true" if a == "--enable-ldw-opt=false" else a
            for a in argv]
    return _orig_run_command(argv, **kw)


_bu.run_command = _run_command_ldwopt

F32 = mybir.dt.float32
F32R = mybir.dt.float32r
BF16 = mybir.dt.bfloat16

P = 128            # partitions
H = 1024           # hidden
N = 2048           # sequence (n1 == n2)
KB = H // P        # 8 k-blocks of 128
MT = N // P        # 16 row tiles of 128
NC4 = N // 512     # 4 column chunks of 512
AFT = mybir.ActivationFunctionType
AXX = mybir.AxisListType.X


class _TC(tile.TileContext):
    """TileContext for a walrus build that accepts at most ONE sync wait per
    instruction (2 on EventSemaphore): splits the final drain's waits, and
    runs a post-pass hoisting excess body waits into EventSemaphore carriers.
    """

    def _cap_waits(self):
        nc = self.nc
        for bbw in nc.bb_map.values():
            bb = bbw.bb
            insts = bb.instructions
            out = []
            changed = False
            for inst in insts:
                si = inst.sync_info
                cap = 2 if inst.opcode == "EventSemaphore" else 1
                if si is not None and len(si.on_wait) > cap:
                    waits = list(si.on_wait)
                    extra, keep = waits[:-cap], waits[-cap:]
                    while extra:
                        batch, extra = extra[:2], extra[2:]
                        carrier = mybir.InstEventSemaphore(
                            name=nc.get_next_instruction_name(),
                            ins=[], outs=[], engine=inst.engine,
                            sync_info=mybir.SyncInfo(on_wait=batch, on_update=[]),
                        )
                        out.append(carrier)
                    inst.sync_info = mybir.SyncInfo(
                        on_wait=keep, on_update=list(si.on_update))
                    changed = True
                out.append(inst)
            if changed:
                bb.instructions = out

    def _drain_and_barrier(self, tick_clock, wait_clock):
        self._cap_waits()
        nc = self.nc
        dummy = mybir.InstDrain(
            name="dummy_drain_waits", ins=[], outs=[], engine=mybir.EngineType.SP
        )
        wait_clock.add_sem_waits(dummy, ScopedClock({None: tick_clock.global_clock}))
        waits = list(dummy.sync_info.on_wait) if dummy.sync_info else []
        handles = {h.name: h for h in self.sems.allocated().values()}
        for w in waits:
            assert w.sync_type == "semaphore", w
            h = handles.get(w.ant_name)
            assert h is not None, (w.ant_name, sorted(handles))
            nc.sync.wait_ge(h, w.wait_value)
        nc.sync.drain()
        nc.all_engine_barrier()
        assert self.sems is not None
        popped = nc._tile_sem_poison_stack.pop()
        assert popped is self._sem_poison
        nc.clear_and_free_semaphores(list(self.sems.allocated().values()))
        nc.all_engine_barrier()


def _r(ap):
    return ap.bitcast(F32R)


def _emit(tc, io, pools, stages="all"):
    nc = tc.nc
    (ident, big_a, big_b, kstat_pool, o1024_pool, attn_pool, aT_pool, st_pool,
     bounce_pool, ptp_pool, ps_pool, po_pool) = pools
    S1, S2, W1, W2, O1, O2, s1T_d, s1wT_d, s2wT_d = io

    def transpose_into(dst3, src2, src_cols_ofs):
        """PE-transpose src2 [P, P] (SBUF) into dst3 slice."""
        pt = ptp_pool.tile([P, P], F32, tag="ptp")
        nc.tensor.transpose(pt[:], src2, ident[:])
        nc.vector.tensor_copy(out=dst3, in_=pt[:])

    do_tw = stages in ("all", "tw")
    do_a = stages in ("all", "a")
    # ---------------- T1: S1 -> s1T in big_a; spill to s1T_d --------------
    for i in range(do_tw and MT or 0):
        xt = o1024_pool.tile([P, H], F32, tag="o1024")
        nc.sync.dma_start(out=xt[:], in_=S1[i * P:(i + 1) * P, :])
        for j in range(KB):
            transpose_into(big_a[:, j, i * P:(i + 1) * P], xt[:, j * P:(j + 1) * P],
                           None)
    for j in range(do_tw and KB or 0):
        nc.scalar.dma_start(out=s1T_d[j * P:(j + 1) * P, :], in_=big_a[:, j, :])

    # ---------------- W1: s1wT = W1(k,h') x s1T -> s1wT_d -----------------
    def w_stage(W, xT, out_d):
        Wv = W.bitcast(F32R).rearrange("(kb p) h -> p kb h", p=P)
        for hb in range(KB):
            wt = kstat_pool.tile([P, KB, P], F32R, tag="kstat")
            nc.sync.dma_start(out=wt[:], in_=Wv[:, :, hb * P:(hb + 1) * P])
            # 1-bank psum chunks on the transpose pool's tag keep the W
            # stage off the ps/po slots, so the next attention stage's
            # scores can overlap this stage's tail
            for mc in range(NC4):
                pw = ptp_pool.tile([P, 512], F32, tag="ptp")
                for kb in range(KB):
                    nc.tensor.matmul(
                        pw[:],
                        lhsT=wt[:, kb, :],
                        rhs=xT[:, kb, mc * 512:(mc + 1) * 512],
                        start=(kb == 0), stop=(kb == KB - 1),
                    )
                bw = bounce_pool.tile([P, 512], F32R, tag="bounce")
                nc.vector.tensor_copy(out=bw[:], in_=pw[:])
                nc.scalar.dma_start(
                    out=out_d[hb * P:(hb + 1) * P, mc * 512:(mc + 1) * 512],
                    in_=bw[:])

    if do_tw:
        w_stage(W1, big_a, s1wT_d)

    # ---------------- T2: S2 natural -> big_b; s2T -> big_a ---------------
    for i in range(do_tw and MT or 0):
        nc.sync.dma_start(out=big_b[:, i, :],
                          in_=S2.bitcast(F32R)[i * P:(i + 1) * P, :])
        for j in range(KB):
            transpose_into(big_a[:, j, i * P:(i + 1) * P],
                           big_b[:, i, j * P:(j + 1) * P].bitcast(F32), None)

    # ---------------- W2 --------------------------------------------------
    if do_tw:
        w_stage(W2, big_a, s2wT_d)

    # ---------------- Attention stage (software-pipelined) ----------------
    # scores+softmax of tile mt are emitted before transposes/AV of tile
    # mt-1, so the softmax latency (DVE max + ACT exp) hides under the
    # previous tile's PE work instead of stalling the PE.
    def a_stage(qwT_d, kT, v, O):
        qv = qwT_d.rearrange("(kb p) m -> p kb m", p=P)

        def scores_softmax(mt):
            qw = kstat_pool.tile([P, KB, P], F32R, tag="kstat")
            nc.sync.dma_start(out=qw[:], in_=qv[:, :, mt * P:(mt + 1) * P])
            ps = ps_pool.tile([P, N], F32, tag="ps")
            cmx = st_pool.tile([P, NC4], F32, tag="cmx")
            for ck in range(NC4):
                for kb in range(KB):
                    nc.tensor.matmul(
                        ps[:, ck * 512:(ck + 1) * 512],
                        lhsT=qw[:, kb, :],
                        rhs=kT[:, kb, ck * 512:(ck + 1) * 512],
                        start=(kb == 0), stop=(kb == KB - 1),
                    )
                # per-chunk max overlaps DVE with the next chunk's matmuls
                nc.vector.reduce_max(out=cmx[:, ck:ck + 1],
                                     in_=ps[:, ck * 512:(ck + 1) * 512], axis=AXX)
            nmx = st_pool.tile([P, 1], F32, tag="st")
            nc.vector.reduce_max(out=nmx[:], in_=cmx[:], axis=AXX, negate=True)
            attn = attn_pool.tile([P, N], F32, tag="attn")
            sumc = st_pool.tile([P, NC4], F32, tag="sumc")
            for ck in range(NC4):
                nc.scalar.activation(attn[:, ck * 512:(ck + 1) * 512],
                                     ps[:, ck * 512:(ck + 1) * 512], AFT.Exp,
                                     bias=nmx[:], accum_out=sumc[:, ck:ck + 1])
            sume = st_pool.tile([P, 1], F32, tag="st")
            nc.vector.reduce_sum(out=sume[:], in_=sumc[:], axis=AXX)
            rec = st_pool.tile([P, 1], F32, tag="st")
            nc.vector.reciprocal(rec[:], sume[:])
            return attn, rec

        def finish(attn, rec, mt):
            aT = aT_pool.tile([P, MT, P], F32R, tag="aT")
            for nt in range(MT):
                transpose_into(aT[:, nt, :], attn[:, nt * P:(nt + 1) * P],
                               None)
            po = po_pool.tile([P, H], F32, tag="po")
            for nt in range(MT):
                for hc in range(2):
                    nc.tensor.matmul(
                        po[:, hc * 512:(hc + 1) * 512],
                        lhsT=aT[:, nt, :],
                        rhs=v[:, nt, hc * 512:(hc + 1) * 512],
                        start=(nt == 0), stop=(nt == MT - 1),
                    )
            ot = o1024_pool.tile([P, H], F32, tag="o1024")
            nc.vector.tensor_scalar_mul(ot[:], po[:], rec[:])
            nc.scalar.dma_start(out=O[mt * P:(mt + 1) * P, :], in_=ot[:])

        prev = None
        for mt in range(MT):
            cur = scores_softmax(mt)
            if prev is not None:
                finish(prev[0], prev[1], mt - 1)
            prev = cur
        finish(prev[0], prev[1], MT - 1)

    if not do_a:
        return
    # A1: queries=S1 rows, keys=s2T (big_a), values=S2 natural (big_b)
    a_stage(s1wT_d, big_a, big_b, O1)

    # R2: reload s1T into big_a, S1 natural into big_b
    for j in range(KB):
        nc.sync.dma_start(out=big_a[:, j, :], in_=s1T_d[j * P:(j + 1) * P, :])
    for i in range(MT):
        nc.sync.dma_start(out=big_b[:, i, :],
                          in_=S1.bitcast(F32R)[i * P:(i + 1) * P, :])

    # A2
    a_stage(s2wT_d, big_a, big_b, O2)


def build(reps=1, loop=None, stages="all"):
    nc = bass.Bass(name="biaffine")
    S1 = nc.dram_tensor("S1", (N, H), F32, kind="ExternalInput")[:]
    S2 = nc.dram_tensor("S2", (N, H), F32, kind="ExternalInput")[:]
    W1 = nc.dram_tensor("W1", (H, H), F32, kind="ExternalInput")[:]
    W2 = nc.dram_tensor("W2", (H, H), F32, kind="ExternalInput")[:]
    O1 = nc.dram_tensor("O1", (N, H), F32, kind="ExternalOutput")[:]
    O2 = nc.dram_tensor("O2", (N, H), F32, kind="ExternalOutput")[:]
    s1T_d = nc.dram_tensor("s1T_sp", (H, N), F32R, kind="Internal")[:]
    s1wT_d = nc.dram_tensor("s1wT_sp", (H, N), F32R, kind="Internal")[:]
    s2wT_d = nc.dram_tensor("s2wT_sp", (H, N), F32R, kind="Internal")[:]
    io = (S1, S2, W1, W2, O1, O2, s1T_d, s1wT_d, s2wT_d)

    with _TC(nc) as tc:
        with tc.tile_pool(name="consts", bufs=1) as consts, \
             tc.tile_pool(name="biga", bufs=1) as biga, \
             tc.tile_pool(name="bigb", bufs=1) as bigb, \
             tc.tile_pool(name="kstat", bufs=2) as kstat_pool, \
             tc.tile_pool(name="o1024", bufs=2) as o1024_pool, \
             tc.tile_pool(name="attn", bufs=2) as attn_pool, \
             tc.tile_pool(name="aTp", bufs=2) as aT_pool, \
             tc.tile_pool(name="st", bufs=16) as st_pool, \
             tc.tile_pool(name="bounce", bufs=2) as bounce_pool, \
             tc.tile_pool(name="ptp", bufs=2, space="PSUM") as ptp_pool, \
             tc.tile_pool(name="ps", bufs=1, space="PSUM") as ps_pool, \
             tc.tile_pool(name="po", bufs=1, space="PSUM") as po_pool:
            ident = consts.tile([P, P], F32)
            masks.make_identity(nc, ident[:])
            big_a = biga.tile([P, KB, N], F32R)
            big_b = bigb.tile([P, MT, H], F32R)
            pools = (ident, big_a, big_b, kstat_pool, o1024_pool, attn_pool,
                     aT_pool, st_pool, bounce_pool, ptp_pool, ps_pool, po_pool)
            if loop is not None:
                with tc.For_i(0, loop, 1):
                    _emit(tc, io, pools, stages)
            else:
                for _ in range(reps):
                    _emit(tc, io, pools, stages)
    return nc


_nc_cache = {}


def _get_nc(reps=1):
    if reps not in _nc_cache:
        _nc_cache[reps] = build(reps)
    return _nc_cache[reps]


def run_on_cores(inputs, reps=1):
    from concourse.bass_utils import run_bass_kernel_spmd

    nc = _get_nc(reps)
    S1 = np.asarray(inputs["S1"], dtype=np.float32)
    S2 = np.asarray(inputs["S2"], dtype=np.float32)
    W1 = np.ascontiguousarray(np.asarray(inputs["W1"], dtype=np.float32))
    W2 = np.ascontiguousarray(np.asarray(inputs["W2"], dtype=np.float32))
    b = S1.shape[0]
    assert b == 8
    in_maps = [
        {
            "S1": np.ascontiguousarray(S1[i]),
            "S2": np.ascontiguousarray(S2[i]),
            "W1": W1,
            "W2": W2,
        }
        for i in range(b)
    ]
    res = run_bass_kernel_spmd(nc, in_maps, core_ids=list(range(b)))
    O1 = np.stack([res.results[i]["O1"] for i in range(b)])
    O2 = np.stack([res.results[i]["O2"] for i in range(b)])
    return O1, O2


def kernel(**inputs):
    O1, O2 = run_on_cores(inputs, reps=1)
    return O1.astype(np.float32), O2.astype(np.float32)



# revision 16
# speedup vs baseline: 1.1076x; 1.1076x over previous
"""BiAffine attention kernel for Trainium2, 8 NeuronCores.

Problem: b=8, n1=n2=2048, h=1024 (fp32)
  S2_h   = S2 @ W1.T ; scores1 = S1 @ S2_h.T ; attn1 = softmax(scores1) ; O1 = attn1 @ S2
  S1_h   = S1 @ W2.T ; scores2 = S2 @ S1_h.T ; attn2 = softmax(scores2) ; O2 = attn2 @ S1

Reformulated (per batch):
  scores1 = (S1 @ W1) @ S2^T        scores2 = (S2 @ W2) @ S1^T

Sharding: data-parallel over batch, 1 batch per core (8 cores).

Per-core plan (all matmuls fp32r = fp22 multiply, fp32 accumulate):
  T1: transpose S1 -> s1T (SBUF), spill s1T to HBM scratch
  W1: s1wT = W1^T-free matmul: s1wT[h',m] = sum_k W1[k,h'] * s1T[k,m] -> HBM
  T2: load S2 natural (resident) + transpose -> s2T (resident)
  W2: s2wT -> HBM
  A1: attention rows, software-pipelined across 128-row tiles: scores in
      PSUM -> chunked softmax (DVE max, ACT exp+rowsum) for tile mt runs
      while the PE does attn-transposes + AV for tile mt-1; output scaled
      by 1/rowsum on the way out.
  R2: reload s1T + S1 natural
  A2: direction 2, symmetric.

Measured (in-NEFF loop diff, 8 cores parallel): ~0.95 ms per full pass.
"""

import sys

sys.path.insert(0, "/opt/trn_rl_repo")

import numpy as np

import concourse.bass as bass
import concourse.tile as tile
import concourse.mybir as mybir
from concourse import masks
from concourse.vector_clock import ScopedClock
import concourse.bass_utils as _bu

_orig_run_command = _bu.run_command


def _run_command_ldwopt(argv, **kw):
    argv = ["--enable-ldw-opt=true" if a == "--enable-ldw-opt=false" else a
            for a in argv]
    return _orig_run_command(argv, **kw)


_bu.run_command = _run_command_ldwopt

F32 = mybir.dt.float32
F32R = mybir.dt.float32r
BF16 = mybir.dt.bfloat16

P = 128            # partitions
H = 1024           # hidden
N = 2048           # sequence (n1 == n2)
KB = H // P        # 8 k-blocks of 128
MT = N // P        # 16 row tiles of 128
NC4 = N // 512     # 4 column chunks of 512
AFT = mybir.ActivationFunctionType
AXX = mybir.AxisListType.X


class _TC(tile.TileContext):
    """TileContext for a walrus build that accepts at most ONE sync wait per
    instruction (2 on EventSemaphore): splits the final drain's waits, and
    runs a post-pass hoisting excess body waits into EventSemaphore carriers.
    """

    def _cap_waits(self):
        nc = self.nc
        for bbw in nc.bb_map.values():
            bb = bbw.bb
            insts = bb.instructions
            out = []
            changed = False
            for inst in insts:
                si = inst.sync_info
                cap = 2 if inst.opcode == "EventSemaphore" else 1
                if si is not None and len(si.on_wait) > cap:
                    waits = list(si.on_wait)
                    extra, keep = waits[:-cap], waits[-cap:]
                    while extra:
                        batch, extra = extra[:2], extra[2:]
                        carrier = mybir.InstEventSemaphore(
                            name=nc.get_next_instruction_name(),
                            ins=[], outs=[], engine=inst.engine,
                            sync_info=mybir.SyncInfo(on_wait=batch, on_update=[]),
                        )
                        out.append(carrier)
                    inst.sync_info = mybir.SyncInfo(
                        on_wait=keep, on_update=list(si.on_update))
                    changed = True
                out.append(inst)
            if changed:
                bb.instructions = out

    def _drain_and_barrier(self, tick_clock, wait_clock):
        self._cap_waits()
        nc = self.nc
        dummy = mybir.InstDrain(
            name="dummy_drain_waits", ins=[], outs=[], engine=mybir.EngineType.SP
        )
        wait_clock.add_sem_waits(dummy, ScopedClock({None: tick_clock.global_clock}))
        waits = list(dummy.sync_info.on_wait) if dummy.sync_info else []
        handles = {h.name: h for h in self.sems.allocated().values()}
        for w in waits:
            assert w.sync_type == "semaphore", w
            h = handles.get(w.ant_name)
            assert h is not None, (w.ant_name, sorted(handles))
            nc.sync.wait_ge(h, w.wait_value)
        nc.sync.drain()
        nc.all_engine_barrier()
        assert self.sems is not None
        popped = nc._tile_sem_poison_stack.pop()
        assert popped is self._sem_poison
        nc.clear_and_free_semaphores(list(self.sems.allocated().values()))
        nc.all_engine_barrier()


def _r(ap):
    return ap.bitcast(F32R)


def _emit(tc, io, pools, stages="all"):
    nc = tc.nc
    (ident, big_a, big_b, kstat_pool, o1024_pool, attn_pool, aT_pool, st_pool,
     bounce_pool, ptp_pool, ps_pool, po_pool) = pools
    S1, S2, W1, W2, O1, O2, s1T_d, s1wT_d, s2wT_d = io
    identR = ident[:]

    def transpose4_into(dst, srcs):
        """PE-transpose four [P, P] f32r blocks into one psum bank, then a
        single DVE copy into dst ([P, 512] or [P, 4, P] slice)."""
        pt = ptp_pool.tile([P, 4 * P], F32R, tag="ptp")
        for t, src in enumerate(srcs):
            nc.tensor.transpose(pt[:, t * P:(t + 1) * P], src, identR)
        nc.vector.tensor_copy(out=dst, in_=pt[:])

    do_tw = stages in ("all", "tw")
    do_a = stages in ("all", "a")
    # ---------------- P0: prefetch S2 natural -> big_b (ACT queue) and ----
    # the first W1 weight tile (SP queue, ahead of the S1 loads)
    Wv1 = W1.bitcast(F32R).rearrange("(kb p) h -> p kb h", p=P)
    Wv2 = W2.bitcast(F32R).rearrange("(kb p) h -> p kb h", p=P)
    wt_first = None
    if do_tw:
        wt_first = kstat_pool.tile([P, KB, P], F32R, tag="kstat")
        nc.sync.dma_start(out=wt_first[:], in_=Wv1[:, :, 0:P])
    for i in range(do_tw and MT or 0):
        nc.scalar.dma_start(out=big_b[:, i, :],
                            in_=S2.bitcast(F32R)[i * P:(i + 1) * P, :])

    # ---------------- T1: S1 -> s1T in big_a; spill to s1T_d --------------
    if do_tw:
        for ig in range(MT // 4):
            xts = []
            for t in range(4):
                xt = o1024_pool.tile([P, H], F32R, tag="o1024")
                i = ig * 4 + t
                nc.sync.dma_start(out=xt[:],
                                  in_=S1.bitcast(F32R)[i * P:(i + 1) * P, :])
                xts.append(xt)
            for j in range(KB):
                transpose4_into(
                    big_a[:, j, ig * 512:(ig + 1) * 512],
                    [xt[:, j * P:(j + 1) * P] for xt in xts])
        for j in range(KB):
            nc.scalar.dma_start(out=s1T_d[j * P:(j + 1) * P, :],
                                in_=big_a[:, j, :])

    # ---------------- W1: s1wT = W1(k,h') x s1T -> s1wT_d -----------------
    def w_stage(Wv, xT, out_d, wt0=None):
        for hb in range(KB):
            if hb == 0 and wt0 is not None:
                wt = wt0
            else:
                wt = kstat_pool.tile([P, KB, P], F32R, tag="kstat")
                nc.sync.dma_start(out=wt[:], in_=Wv[:, :, hb * P:(hb + 1) * P])
            # 1-bank psum chunks on the transpose pool's tag keep the W
            # stage off the ps/po slots, so the next attention stage's
            # scores can overlap this stage's tail
            for mc in range(NC4):
                pw = ptp_pool.tile([P, 512], F32, tag="ptp")
                for kb in range(KB):
                    nc.tensor.matmul(
                        pw[:],
                        lhsT=wt[:, kb, :],
                        rhs=xT[:, kb, mc * 512:(mc + 1) * 512],
                        start=(kb == 0), stop=(kb == KB - 1),
                    )
                bw = bounce_pool.tile([P, 512], F32R, tag="bounce")
                nc.vector.tensor_copy(out=bw[:], in_=pw[:])
                nc.scalar.dma_start(
                    out=out_d[hb * P:(hb + 1) * P, mc * 512:(mc + 1) * 512],
                    in_=bw[:])

    if do_tw:
        w_stage(Wv1, big_a, s1wT_d, wt0=wt_first)

    # ---------------- T2: s2T -> big_a (S2 natural already prefetched) ----
    for ig in range(do_tw and MT // 4 or 0):
        for j in range(KB):
            transpose4_into(
                big_a[:, j, ig * 512:(ig + 1) * 512],
                [big_b[:, ig * 4 + t, j * P:(j + 1) * P] for t in range(4)])

    # ---------------- W2 --------------------------------------------------
    if do_tw:
        w_stage(Wv2, big_a, s2wT_d)

    # ---------------- Attention stage (software-pipelined) ----------------
    # scores+softmax of tile mt are emitted before transposes/AV of tile
    # mt-1, so the softmax latency (DVE max + ACT exp) hides under the
    # previous tile's PE work instead of stalling the PE.
    def a_stage(qwT_d, kT, v, O):
        qv = qwT_d.rearrange("(kb p) m -> p kb m", p=P)

        def scores_softmax(mt):
            qw = kstat_pool.tile([P, KB, P], F32R, tag="kstat")
            nc.sync.dma_start(out=qw[:], in_=qv[:, :, mt * P:(mt + 1) * P])
            ps = ps_pool.tile([P, N], F32, tag="ps")
            cmx = st_pool.tile([P, NC4], F32, tag="cmx")
            for ck in range(NC4):
                for kb in range(KB):
                    nc.tensor.matmul(
                        ps[:, ck * 512:(ck + 1) * 512],
                        lhsT=qw[:, kb, :],
                        rhs=kT[:, kb, ck * 512:(ck + 1) * 512],
                        start=(kb == 0), stop=(kb == KB - 1),
                    )
                # per-chunk max overlaps DVE with the next chunk's matmuls
                nc.vector.reduce_max(out=cmx[:, ck:ck + 1],
                                     in_=ps[:, ck * 512:(ck + 1) * 512], axis=AXX)
            nmx = st_pool.tile([P, 1], F32, tag="st")
            nc.vector.reduce_max(out=nmx[:], in_=cmx[:], axis=AXX, negate=True)
            attn = attn_pool.tile([P, N], F32R, tag="attn")
            sumc = st_pool.tile([P, NC4], F32, tag="sumc")
            for ck in range(NC4):
                nc.scalar.activation(attn[:, ck * 512:(ck + 1) * 512],
                                     ps[:, ck * 512:(ck + 1) * 512], AFT.Exp,
                                     bias=nmx[:], accum_out=sumc[:, ck:ck + 1])
            sume = st_pool.tile([P, 1], F32, tag="st")
            nc.vector.reduce_sum(out=sume[:], in_=sumc[:], axis=AXX)
            rec = st_pool.tile([P, 1], F32, tag="st")
            nc.vector.reciprocal(rec[:], sume[:])
            return attn, rec

        def finish(attn, rec, mt):
            aT = aT_pool.tile([P, MT, P], F32R, tag="aT")
            for ng in range(MT // 4):
                transpose4_into(
                    aT[:, ng * 4:(ng + 1) * 4, :],
                    [attn[:, (ng * 4 + t) * P:(ng * 4 + t + 1) * P]
                     for t in range(4)])
            po = po_pool.tile([P, H], F32, tag="po")
            for nt in range(MT):
                for hc in range(2):
                    nc.tensor.matmul(
                        po[:, hc * 512:(hc + 1) * 512],
                        lhsT=aT[:, nt, :],
                        rhs=v[:, nt, hc * 512:(hc + 1) * 512],
                        start=(nt == 0), stop=(nt == MT - 1),
                    )
            ot = o1024_pool.tile([P, H], F32, tag="o1024")
            nc.vector.tensor_scalar_mul(ot[:], po[:], rec[:])
            nc.scalar.dma_start(out=O[mt * P:(mt + 1) * P, :], in_=ot[:])

        prev = None
        for mt in range(MT):
            cur = scores_softmax(mt)
            if prev is not None:
                finish(prev[0], prev[1], mt - 1)
            prev = cur
        finish(prev[0], prev[1], MT - 1)

    if not do_a:
        return
    # A1: queries=S1 rows, keys=s2T (big_a), values=S2 natural (big_b)
    a_stage(s1wT_d, big_a, big_b, O1)

    # R2: reload s1T into big_a, S1 natural into big_b
    for j in range(KB):
        nc.sync.dma_start(out=big_a[:, j, :], in_=s1T_d[j * P:(j + 1) * P, :])
    for i in range(MT):
        nc.sync.dma_start(out=big_b[:, i, :],
                          in_=S1.bitcast(F32R)[i * P:(i + 1) * P, :])

    # A2
    a_stage(s2wT_d, big_a, big_b, O2)


def build(reps=1, loop=None, stages="all"):
    nc = bass.Bass(name="biaffine")
    S1 = nc.dram_tensor("S1", (N, H), F32, kind="ExternalInput")[:]
    S2 = nc.dram_tensor("S2", (N, H), F32, kind="ExternalInput")[:]
    W1 = nc.dram_tensor("W1", (H, H), F32, kind="ExternalInput")[:]
    W2 = nc.dram_tensor("W2", (H, H), F32, kind="ExternalInput")[:]
    O1 = nc.dram_tensor("O1", (N, H), F32, kind="ExternalOutput")[:]
    O2 = nc.dram_tensor("O2", (N, H), F32, kind="ExternalOutput")[:]
    s1T_d = nc.dram_tensor("s1T_sp", (H, N), F32R, kind="Internal")[:]
    s1wT_d = nc.dram_tensor("s1wT_sp", (H, N), F32R, kind="Internal")[:]
    s2wT_d = nc.dram_tensor("s2wT_sp", (H, N), F32R, kind="Internal")[:]
    io = (S1, S2, W1, W2, O1, O2, s1T_d, s1wT_d, s2wT_d)

    with _TC(nc) as tc:
        with tc.tile_pool(name="consts", bufs=1) as consts, \
             tc.tile_pool(name="biga", bufs=1) as biga, \
             tc.tile_pool(name="bigb", bufs=1) as bigb, \
             tc.tile_pool(name="kstat", bufs=2) as kstat_pool, \
             tc.tile_pool(name="o1024", bufs=4) as o1024_pool, \
             tc.tile_pool(name="attn", bufs=2) as attn_pool, \
             tc.tile_pool(name="aTp", bufs=2) as aT_pool, \
             tc.tile_pool(name="st", bufs=16) as st_pool, \
             tc.tile_pool(name="bounce", bufs=2) as bounce_pool, \
             tc.tile_pool(name="ptp", bufs=2, space="PSUM") as ptp_pool, \
             tc.tile_pool(name="ps", bufs=1, space="PSUM") as ps_pool, \
             tc.tile_pool(name="po", bufs=1, space="PSUM") as po_pool:
            ident32 = consts.tile([P, P], F32)
            masks.make_identity(nc, ident32[:])
            ident = consts.tile([P, P], F32R)
            nc.vector.tensor_copy(out=ident[:], in_=ident32[:])
            big_a = biga.tile([P, KB, N], F32R)
            big_b = bigb.tile([P, MT, H], F32R)
            pools = (ident, big_a, big_b, kstat_pool, o1024_pool, attn_pool,
                     aT_pool, st_pool, bounce_pool, ptp_pool, ps_pool, po_pool)
            if loop is not None:
                with tc.For_i(0, loop, 1):
                    _emit(tc, io, pools, stages)
            else:
                for _ in range(reps):
                    _emit(tc, io, pools, stages)
    return nc


_nc_cache = {}


def _get_nc(reps=1):
    if reps not in _nc_cache:
        _nc_cache[reps] = build(reps)
    return _nc_cache[reps]


def run_on_cores(inputs, reps=1):
    from concourse.bass_utils import run_bass_kernel_spmd

    nc = _get_nc(reps)
    S1 = np.asarray(inputs["S1"], dtype=np.float32)
    S2 = np.asarray(inputs["S2"], dtype=np.float32)
    W1 = np.ascontiguousarray(np.asarray(inputs["W1"], dtype=np.float32))
    W2 = np.ascontiguousarray(np.asarray(inputs["W2"], dtype=np.float32))
    b = S1.shape[0]
    assert b == 8
    in_maps = [
        {
            "S1": np.ascontiguousarray(S1[i]),
            "S2": np.ascontiguousarray(S2[i]),
            "W1": W1,
            "W2": W2,
        }
        for i in range(b)
    ]
    res = run_bass_kernel_spmd(nc, in_maps, core_ids=list(range(b)))
    O1 = np.stack([res.results[i]["O1"] for i in range(b)])
    O2 = np.stack([res.results[i]["O2"] for i in range(b)])
    return O1, O2


def kernel(**inputs):
    O1, O2 = run_on_cores(inputs, reps=1)
    return O1.astype(np.float32), O2.astype(np.float32)

